# revision 41
# baseline (speedup 1.0000x reference)
"""AriaTextMoELayer on 8 TRN2 NeuronCores — expert-parallel with real
token dispatch.

Sharding strategy (hardcoded for E=8 experts, TOPK=2, H=1024, I=1024,
ISH=2048, B*S = 2048 tokens, 8 cores):
  - The router (logits -> top-2 -> softmax) runs on host as part of
    input sharding: tokens are dispatched (all-to-all style) so core e
    receives exactly the tokens routed to expert e (zero-padded to a
    common capacity `cap`), pre-transposed into device tile layout.
  - Core e owns expert e's fc1/fc2 and runs the SwiGLU MLP densely over
    its ~cap gathered tokens (vs 2048 dense) — 4x less expert FLOPs.
  - Shared-expert MLP is token-parallel: core e runs the full shared
    SwiGLU for tokens [256e, 256e+256) with replicated gate/up/down.
    It is computed FIRST on device (needs only 1MB of DMA to start)
    while the expert weights stream in behind it.
  - No collectives. Host un-shards: out[tok] = sum_k w_k * yg_ek[tok]
    (router-weighted scatter-add) + shared slice.

All host->device tensors are pre-shuffled on host into the exact SBUF
tile layout ([128 partitions, ktile, cols], proj/gate and gate/up pairs
interleaved per 128-col group) so every DMA is a contiguous full-BW
block copy and each 0.5MB chunk unlocks one SwiGLU pair of compute.
"""
import sys

if "/opt/trn_rl_repo" not in sys.path:
    sys.path.insert(0, "/opt/trn_rl_repo")

import numpy as np

from concourse import bacc, bass, mybir, tile

E = 8
TOPK = 2
H = 1024
I = 1024
I2 = 2048          # 2*I (fc1 output: [proj | gate])
ISH = 2048         # shared intermediate
N = 2048           # tokens
SSL = 256          # shared-token slice per core
NCORES = 8
KT = H // 128      # 8 contraction tiles over H
IT = I // 128      # 8 contraction tiles over I
ST = ISH // 128    # 16 tiles over shared intermediate

F32 = mybir.dt.float32
BF16 = mybir.dt.bfloat16
OP = mybir.AluOpType
ACTF = mybir.ActivationFunctionType


def _chunks(n, c=512):
    out = []
    s = 0
    while s < n:
        out.append((s, min(s + c, n)))
        s += c
    return out


def build(cap):
    nc = bacc.Bacc(None, target_bir_lowering=False, debug=False)

    xg_d = nc.declare_dram_parameter("xg", [128, KT, cap], BF16, isOutput=False)
    xs_d = nc.declare_dram_parameter("xs", [128, KT, SSL], BF16, isOutput=False)
    fc1_d = nc.declare_dram_parameter(
        "fc1", [IT, 128, KT, 256], BF16, isOutput=False
    )
    fc2_d = nc.declare_dram_parameter("fc2", [128, IT, H], BF16, isOutput=False)
    gwu_d = nc.declare_dram_parameter(
        "gwu", [ST, 128, KT, 256], BF16, isOutput=False
    )
    dw_d = nc.declare_dram_parameter("dw", [128, ST, H], BF16, isOutput=False)
    yg_d = nc.declare_dram_parameter("yg", [cap, H], BF16, isOutput=True)
    ys_d = nc.declare_dram_parameter("ys", [SSL, H], BF16, isOutput=True)
    scr_d = nc.declare_dram_parameter("scr", [1, 16], BF16, isOutput=True)

    nt = -(-cap // 128)  # token tiles for expert GEMM2

    with tile.TileContext(nc) as tc:
        with (
            tc.tile_pool(name="wpool", bufs=1) as wpool,
            tc.tile_pool(name="xpool", bufs=1) as xpool,
            tc.tile_pool(name="gpool", bufs=1) as gpool,
            tc.tile_pool(name="tmppool", bufs=3) as tmppool,
            tc.tile_pool(name="stpool", bufs=4) as stpool,
            tc.tile_pool(name="psab", bufs=4, space="PSUM") as psab,
            tc.tile_pool(name="psey", bufs=4, space="PSUM") as psey,
        ):
            # ---- HAM warm-up: ~10 dummy matmuls fill the otherwise-idle
            # window between the kernel preamble and the first input DMA
            # landing, so the PE clock gate is at 8/8 when real work
            # starts (tiny DMA tie-off keeps DCE away). ----
            wup = wpool.tile([128, 512], BF16)
            nc.vector.memset(wup[:], 0.0)
            wp = psab.tile([128, 512], F32, tag="ab")
            for r in range(12):
                nc.tensor.matmul(
                    wp[:], wup[:, 0:128], wup[:], start=(r == 0), stop=(r == 11)
                )
            stw = stpool.tile([1, 16], BF16, tag="stw")
            nc.vector.tensor_copy(stw[:], wp[0:1, 0:16])

            # ---- DMAs (emission order = fetch priority, single HWDGE
            # queue: HBM BW is shared, so strict priority order beats
            # parallel queues). First chunks k-sliced so the first
            # matmul starts ASAP. ----
            xs_t = xpool.tile([128, KT, SSL], BF16)
            nc.sync.dma_start(xs_t[:, 0:4, :], xs_d[:, 0:4, :])
            gwu_t = wpool.tile([128, ST, KT, 256], BF16)
            nc.sync.dma_start(gwu_t[:, 0, 0:4], gwu_d[0, :, 0:4])
            nc.sync.dma_start(xs_t[:, 4:8, :], xs_d[:, 4:8, :])
            nc.sync.dma_start(gwu_t[:, 0, 4:8], gwu_d[0, :, 4:8])
            for o in range(1, ST):
                nc.sync.dma_start(gwu_t[:, o], gwu_d[o])
            xg_t = xpool.tile([128, KT, cap], BF16)
            nc.sync.dma_start(xg_t[:], xg_d[:])
            fc1_t = wpool.tile([128, IT, KT, 256], BF16)
            for j in range(IT):
                nc.sync.dma_start(fc1_t[:, j], fc1_d[j])
            dw_t = wpool.tile([128, ST, H], BF16)
            for k0 in range(0, ST, 8):
                nc.sync.dma_start(
                    dw_t[:, k0 : k0 + 8, :], dw_d[:, k0 : k0 + 8, :]
                )
            fc2_t = wpool.tile([128, IT, H], BF16)
            for k0 in range(0, IT, 4):
                nc.sync.dma_start(
                    fc2_t[:, k0 : k0 + 4, :], fc2_d[:, k0 : k0 + 4, :]
                )

            # ---- shared GEMM1 + SwiGLU -> sh_t [128, ST(i), SSL] bf16 ----
            sh_t = gpool.tile([128, ST, SSL], BF16)
            for o in range(ST):  # 16 gate/up 128-col pairs
                pg = psab.tile([128, SSL], F32, tag="ab")
                for k in range(KT):
                    nc.tensor.matmul(
                        pg[:],
                        gwu_t[:, o, k, 0:128],
                        xs_t[:, k, :],
                        start=(k == 0),
                        stop=(k == KT - 1),
                    )
                pu = psab.tile([128, SSL], F32, tag="ab")
                for k in range(KT):
                    nc.tensor.matmul(
                        pu[:],
                        gwu_t[:, o, k, 128:256],
                        xs_t[:, k, :],
                        start=(k == 0),
                        stop=(k == KT - 1),
                    )
                stmp = tmppool.tile([128, SSL], F32, tag="silu")
                nc.scalar.activation(stmp[:], pg[:], ACTF.Silu)
                nc.vector.tensor_tensor(
                    sh_t[:, o, :], stmp[:], pu[:], OP.mult
                )

            # ---- shared down tile (interleaved into expert GEMM1: it
            # needs no new DMA, so it absorbs fc1/xg arrival jitter) ----
            def down_tile(t):
                t0 = t * 128
                pd0 = psey.tile([128, 512], F32, tag="ey")
                pd1 = psey.tile([128, 512], F32, tag="ey")
                for i in range(ST):
                    nc.tensor.matmul(
                        pd0[:],
                        sh_t[:, i, t0 : t0 + 128],
                        dw_t[:, i, 0:512],
                        start=(i == 0),
                        stop=(i == ST - 1),
                    )
                    nc.tensor.matmul(
                        pd1[:],
                        sh_t[:, i, t0 : t0 + 128],
                        dw_t[:, i, 512:1024],
                        start=(i == 0),
                        stop=(i == ST - 1),
                    )
                st0 = stpool.tile([128, 512], BF16, tag="st")
                nc.vector.tensor_copy(st0[:], pd0[:])
                nc.sync.dma_start(ys_d[t0 : t0 + 128, 0:512], st0[:])
                st1 = stpool.tile([128, 512], BF16, tag="st")
                nc.vector.tensor_copy(st1[:], pd1[:])
                nc.sync.dma_start(ys_d[t0 : t0 + 128, 512:1024], st1[:])

            # ---- expert GEMM1 + SwiGLU -> g_t [128, IT(i), cap] bf16 ----
            g_t = gpool.tile([128, IT, cap], BF16)
            for j in range(IT):  # 8 proj/gate 128-col pairs
                if j == 3:
                    down_tile(0)
                elif j == 5:
                    down_tile(1)
                for ts, te in _chunks(cap):
                    csz = te - ts
                    pa = psab.tile([128, csz], F32, tag="ab")
                    for k in range(KT):
                        nc.tensor.matmul(
                            pa[:],
                            fc1_t[:, j, k, 0:128],
                            xg_t[:, k, ts:te],
                            start=(k == 0),
                            stop=(k == KT - 1),
                        )
                    pb = psab.tile([128, csz], F32, tag="ab")
                    for k in range(KT):
                        nc.tensor.matmul(
                            pb[:],
                            fc1_t[:, j, k, 128:256],
                            xg_t[:, k, ts:te],
                            start=(k == 0),
                            stop=(k == KT - 1),
                        )
                    stmp = tmppool.tile([128, csz], F32, tag="silu")
                    nc.scalar.activation(stmp[:], pa[:], ACTF.Silu)
                    nc.vector.tensor_tensor(
                        g_t[:, j, ts:te], stmp[:], pb[:], OP.mult
                    )

            # ---- expert GEMM2: yg[t, :] = g_t[:, :, t].T @ fc2 ----
            # (last phase: its final tile is the M=32 remainder, so the
            # post-matmul copy+DMA tail is the shortest available)
            for t in range(nt):
                t0 = t * 128
                rows = min(128, cap - t0)
                pe0 = psey.tile([rows, 512], F32, tag="ey")
                pe1 = psey.tile([rows, 512], F32, tag="ey")
                for i in range(IT):
                    nc.tensor.matmul(
                        pe0[:],
                        g_t[:, i, t0 : t0 + rows],
                        fc2_t[:, i, 0:512],
                        start=(i == 0),
                        stop=(i == IT - 1),
                    )
                    nc.tensor.matmul(
                        pe1[:],
                        g_t[:, i, t0 : t0 + rows],
                        fc2_t[:, i, 512:1024],
                        start=(i == 0),
                        stop=(i == IT - 1),
                    )
                st0 = stpool.tile([rows, 512], BF16, tag="st")
                nc.vector.tensor_copy(st0[:], pe0[:])
                nc.sync.dma_start(yg_d[t0 : t0 + rows, 0:512], st0[:])
                st1 = stpool.tile([rows, 512], BF16, tag="st")
                nc.vector.tensor_copy(st1[:], pe1[:])
                nc.sync.dma_start(yg_d[t0 : t0 + rows, 512:1024], st1[:])

            # warm-up DCE tie-off DMA, emitted LAST so it doesn't block
            # the input stream at the head of the sync queue
            nc.sync.dma_start(scr_d[:], stw[:])

    nc.compile()
    return nc


_CACHED = {}


def _route(x, w_router):
    """Host router: top-2 indices (ties -> lower index, like lax.top_k)
    and softmax weights over the top-2 logits."""
    logits = x.astype(np.float32) @ w_router.astype(np.float32)  # [N, E]
    top2 = np.argsort(-logits, axis=1, kind="stable")[:, :TOPK]  # [N, 2]
    l2 = np.take_along_axis(logits, top2, axis=1)
    m = l2.max(axis=1, keepdims=True)
    ex = np.exp(l2 - m)
    w = ex / ex.sum(axis=1, keepdims=True)
    return top2, w


def _km(a):
    """[H_like, C] -> [128, H_like//128, C] k-major contiguous."""
    kt = a.shape[0] // 128
    return np.ascontiguousarray(a.reshape(kt, 128, a.shape[1]).transpose(1, 0, 2))


def _prep(hidden_states, w_router, fc1_w, fc2_w, gate_w, up_w, down_w):
    import ml_dtypes

    bf16 = ml_dtypes.bfloat16
    x = np.ascontiguousarray(hidden_states.reshape(-1, H), dtype=np.float32)
    top2, w = _route(x, w_router)

    tok_lists = []
    wt_lists = []
    for e in range(NCORES):
        sel = np.where((top2[:, 0] == e) | (top2[:, 1] == e))[0]
        tok_lists.append(sel)
        wt_lists.append(np.where(top2[sel, 0] == e, w[sel, 0], w[sel, 1]))
    max_cnt = max(len(s) for s in tok_lists)
    cap = max(128, -(-max_cnt // 16) * 16)

    xb = x.astype(bf16)
    # gate/up 128-col pairs, shared across cores: [ST, 128, KT, 256]
    gu = np.empty((ST, 128, KT, 256), dtype=bf16)
    gwb = gate_w.astype(bf16)
    uwb = up_w.astype(bf16)
    for o in range(ST):
        gu[o, :, :, 0:128] = _km(gwb[:, o * 128 : (o + 1) * 128])
        gu[o, :, :, 128:256] = _km(uwb[:, o * 128 : (o + 1) * 128])
    dwk = _km(down_w.astype(bf16))  # [128, ST, H]

    in_maps = []
    for e in range(NCORES):
        sel = tok_lists[e]
        xgT = np.zeros((H, cap), dtype=bf16)
        xgT[:, : len(sel)] = xb[sel].T
        f1b = fc1_w[e].astype(bf16)
        f1 = np.empty((IT, 128, KT, 256), dtype=bf16)
        for j in range(IT):
            f1[j, :, :, 0:128] = _km(f1b[:, j * 128 : (j + 1) * 128])
            f1[j, :, :, 128:256] = _km(f1b[:, 1024 + j * 128 : 1024 + (j + 1) * 128])
        in_maps.append(
            {
                "xg": _km(xgT),
                "xs": _km(np.ascontiguousarray(xb[e * SSL : (e + 1) * SSL].T)),
                "fc1": f1,
                "fc2": _km(fc2_w[e].astype(bf16)),
                "gwu": gu,
                "dw": dwk,
            }
        )
    return cap, in_maps, tok_lists, wt_lists


def _assemble(results, tok_lists, wt_lists, orig_shape):
    out = np.zeros((N, H), dtype=np.float32)
    for e, res in enumerate(results):
        out[e * SSL : (e + 1) * SSL] = np.asarray(res["ys"]).astype(np.float32)
    for e, res in enumerate(results):
        sel = tok_lists[e]
        if len(sel) == 0:
            continue
        yg = np.asarray(res["yg"])[: len(sel)].astype(np.float32)
        out[sel] += wt_lists[e][:, None] * yg
    return out.reshape(orig_shape)


def kernel(hidden_states, w_router, fc1_w, fc2_w, gate_w, up_w, down_w):
    from concourse.bass_utils import run_bass_kernel_spmd

    cap, in_maps, tok_lists, wt_lists = _prep(
        hidden_states, w_router, fc1_w, fc2_w, gate_w, up_w, down_w
    )
    if cap not in _CACHED:
        _CACHED[cap] = build(cap)
    nc = _CACHED[cap]
    res = run_bass_kernel_spmd(nc, in_maps, core_ids=list(range(NCORES)))
    return _assemble(res.results, tok_lists, wt_lists, hidden_states.shape)
